# revision 15
# baseline (speedup 1.0000x reference)
"""ConvSP (SPH smoothing-kernel convolution) Trainium2 Bass kernel.

Math (per batch b):
  out[o,i] = bias[o] + sum_k sum_j AT_k[j,o] * relu(r^2 - |x_i - x_j + off_k|^2)^3
  AT_k[j,o] = knorm * sum_c weight[o,c,k] * dcoef[c,j],  dcoef = data/(invmass*density)

Device strategy (8 cores = 2 batches x 4 i-blocks of 512), dense over all
(cell k, j-chunk c) pairs:
  t_k[j,i] = r2 - |x_i - x_j + off_k|^2 is a rank-4 bilinear form:
      V_k[:,j] = [2x_j, 2y_j, 1, 2 x_j.off_k - |x_j|^2]            (lhsT, K=4)
      U_k[:,i] = [x_i, y_i, r2 - |off_k|^2 - |x_i|^2 - 2 x_i.off_k, 1]
  so each [128j x 512i] tile of t is ONE fp32 matmul. relu(t)^3 is computed
  as max(t,0)*t^2 (ACT square + DVE select-mult, both -> bf16), and a bf16
  matmul accumulates out[o,i] += AT_k[j,o].T @ w[j,i] into PSUM over (k, c).

The launch path is latency-dominated (~85ms fixed per dispatch through the
axon-tunneled PJRT backend, plus ~7ms/MB of host->device input traffic), so
inputs are shipped in a compact row form (~0.5MB/core) and expanded into the
matmul layouts on-device:
  VR [12, N]  f32 : rows (2x, 2y, ones, then per-cell 2 off_k.x_j - n2_j)
  UR [12, IW] f32 : rows (x_i, y_i, ones, then per-cell r2-|off_k|^2-n2_i-s_ki)
  DC [64, N]  bf16: dcoef (AT is built on-device with 16 small matmuls)
  WK [64, 9*64] bf16: knorm * weight transposed to [c_in, (k, c_out)]
The jitted PJRT executable is cached so repeat launches skip retrace/compile.
"""

import os
import sys
import time

import numpy as np

for _p in ("/opt/trn_rl_repo", "/root/.axon_site/_ro/trn_rl_repo"):
    if os.path.isdir(_p) and _p not in sys.path:
        sys.path.append(_p)

import ml_dtypes  # noqa: E402

import concourse.bass as bass  # noqa: E402
import concourse.mybir as mybir  # noqa: E402
import concourse.tile as tile  # noqa: E402

# ---------------------------------------------------------------- constants
NDIM = 2
KSIZE = (3, 3)
DILATION = (0.05, 0.05)
RADIUS = 0.1
C_IN = 64
C_OUT = 64
B = 2
N = 2048
NCELLS = 9
R2 = RADIUS * RADIUS
KNORM = 315.0 / (64.0 * np.pi * RADIUS**9)

NCORES = 8
IBLK = 512          # i-columns per core
NCHUNK = N // 128   # 16 j-chunks of 128

F32 = mybir.dt.float32
BF16 = mybir.dt.bfloat16

_cache: dict = {}


# ------------------------------------------------- TileContext drain patch
# The walrus in this container rejects the Tile tail-drain when it carries
# more than ~2 sem waits ("Too many sync wait commands"). Split the waits
# over extra sync-engine NOPs, one wait each.
def _patch_tile_drain():
    if getattr(tile.TileContext, "_drain_patched", False):
        return
    import bass_rust
    from concourse.vector_clock import ScopedClock

    def _drain_and_barrier(self, tick_clock, wait_clock):
        drain_inst = self.nc.sync.drain()
        wait_clock.add_sem_waits(
            drain_inst.ins, ScopedClock({None: tick_clock.global_clock})
        )
        si = drain_inst.ins.sync_info
        waits = list(si.on_wait) if si is not None else []
        if len(waits) > 1:
            si.on_wait = waits[:1]
            drain_inst.ins.sync_info = si
            for w in waits[1:]:
                n = self.nc.sync.nop(nofuse=True, hint="drain_wait_split")
                n.ins.sync_info = bass_rust.SyncInfo(on_wait=[w], on_update=[])
        self.nc.all_engine_barrier()
        popped = self.nc._tile_sem_poison_stack.pop()
        assert popped is self._sem_poison
        self.nc.clear_and_free_semaphores(list(self.sems.allocated().values()))
        self.nc.all_engine_barrier()

    tile.TileContext._drain_and_barrier = _drain_and_barrier
    tile.TileContext._drain_patched = True


# --------------------------------------------- sync-wait legalization pass
# This walrus rejects instructions carrying more than ~1-2 sem waits. After
# Tile scheduling, move excess waits onto same-engine NoOps inserted right
# before the over-subscribed instruction (engines execute their stream in
# order, so semantics are identical).
_WAIT_LIMIT = 1


def _split_sync_waits(nc, limit=_WAIT_LIMIT):
    cnt = 0
    for f in nc.m.functions:
        for bb in f.blocks:
            changed = False
            out = []
            for inst in bb.instructions:
                si = inst.sync_info
                waits = list(si.on_wait) if si is not None else []
                if len(waits) > limit:
                    keep = waits[-limit:]
                    excess = waits[:-limit]
                    for j in range(0, len(excess), limit):
                        n = mybir.InstNoOp(
                            name=f"waitsplit_{cnt}",
                            engine=inst.engine,
                            ins=[],
                            outs=[],
                            sync_info=mybir.SyncInfo(
                                on_wait=excess[j : j + limit], on_update=[]
                            ),
                        )
                        cnt += 1
                        nc.register_instruction(n, overwrite=True)
                        out.append(n)
                    si.on_wait = keep
                    inst.sync_info = si
                    changed = True
                out.append(inst)
            if changed:
                bb.instructions = out
    return cnt


# ------------------------------------------------------------- device build
def _build_nc():
    _patch_tile_drain()

    nc = bass.Bass()
    xy_d = nc.declare_dram_parameter("XY", [2, N], F32, isOutput=False)
    xyi_d = nc.declare_dram_parameter("XYI", [2, IBLK], F32, isOutput=False)
    dc_d = nc.declare_dram_parameter("DC", [C_IN, N], BF16, isOutput=False)
    wk_d = nc.declare_dram_parameter("WK", [C_IN, NCELLS * C_OUT], BF16,
                                     isOutput=False)
    out_d = nc.declare_dram_parameter("out", [C_OUT, IBLK], BF16,
                                      isOutput=True)

    offs = _offsets()

    from contextlib import ExitStack

    with tile.TileContext(nc) as tc, ExitStack() as ctx:
        const = ctx.enter_context(tc.tile_pool(name="const", bufs=1))
        wpool = ctx.enter_context(tc.tile_pool(name="w", bufs=4))
        qpool = ctx.enter_context(tc.tile_pool(name="q", bufs=4))
        opool = ctx.enter_context(tc.tile_pool(name="o", bufs=1, space="PSUM"))

        # ---- load compact inputs & expand to matmul layouts
        dc_t = const.tile([C_IN, N], BF16)
        nc.sync.dma_start(dc_t[:], dc_d[:])
        wk_t = const.tile([C_IN, NCELLS * C_OUT], BF16)
        nc.sync.dma_start(wk_t[:], wk_d[:])

        # Build the bilinear-form factor rows on-device:
        #   V_k rows (j): [2x, 2y, 1, 2 off_k.x - n2]
        #   U_k rows (i): [x, y, r2 - |off_k|^2 - n2 - 2 off_k.x, 1]
        # Compute engines need partition-0-aligned APs, so all row math runs
        # on partition-0 scratch and lands in v_t/u_t partitions 1..3 via DMA.
        v_t = const.tile([4, NCELLS * N], F32)
        u_t = const.tile([4, NCELLS * IBLK], F32)
        mult, add, sub = (mybir.AluOpType.mult, mybir.AluOpType.add,
                          mybir.AluOpType.subtract)

        rows = const.tile([1, 4 * N + 3 * IBLK], F32)   # 1-partition scratch
        r2y = rows[:, 0:N]
        rn2 = rows[:, N : 2 * N]
        rtmp = rows[:, 2 * N : 3 * N]
        ones = rows[:, 3 * N : 4 * N]
        r2xi = rows[:, 4 * N : 4 * N + IBLK]
        r2yi = rows[:, 4 * N + IBLK : 4 * N + 2 * IBLK]
        rn2i = rows[:, 4 * N + 2 * IBLK : 4 * N + 3 * IBLK]
        vstage = const.tile([1, N + 2 * IBLK], F32)
        v3s = vstage[:, 0:N]
        u2a = vstage[:, N : N + IBLK]
        u2b = vstage[:, N + IBLK : N + 2 * IBLK]
        r2x = v_t[0:1, 0:N]                             # partition 0: direct

        with tc.tile_pool(name="xy", bufs=1) as xypool:
            xr = xypool.tile([1, N], F32)
            yr = xypool.tile([1, N], F32)
            xir = xypool.tile([1, IBLK], F32)
            yir = xypool.tile([1, IBLK], F32)
            nc.sync.dma_start(xr[:], xy_d[0:1, :])
            nc.sync.dma_start(yr[:], xy_d[1:2, :])
            nc.sync.dma_start(xir[:], xyi_d[0:1, :])
            nc.sync.dma_start(yir[:], xyi_d[1:2, :])

            nc.scalar.mul(r2x, xr[:], 2.0)
            nc.scalar.mul(r2y, yr[:], 2.0)
            nc.vector.scalar_tensor_tensor(rn2, xr[:], 1.0, xr[:],
                                           op0=mult, op1=mult)
            nc.vector.scalar_tensor_tensor(rtmp, yr[:], 1.0, yr[:],
                                           op0=mult, op1=mult)
            nc.vector.tensor_add(rn2, rn2, rtmp)        # n2 = x^2 + y^2
            nc.scalar.activation(ones, rn2,
                                 mybir.ActivationFunctionType.Copy,
                                 bias=1.0, scale=0.0)
            nc.scalar.mul(r2xi, xir[:], 2.0)
            nc.scalar.mul(r2yi, yir[:], 2.0)
            nc.vector.scalar_tensor_tensor(rn2i, xir[:], 1.0, xir[:],
                                           op0=mult, op1=mult)
            nc.vector.scalar_tensor_tensor(rtmp[:, 0:IBLK], yir[:], 1.0,
                                           yir[:], op0=mult, op1=mult)
            nc.vector.tensor_add(rn2i, rn2i, rtmp[:, 0:IBLK])   # n2_i

            for k in range(NCELLS):
                ox, oy = float(offs[k, 0]), float(offs[k, 1])
                kv = slice(k * N, (k + 1) * N)
                ku = slice(k * IBLK, (k + 1) * IBLK)
                if k > 0:
                    nc.sync.dma_start(v_t[0:1, kv], r2x)
                nc.sync.dma_start(v_t[1:2, kv], r2y)
                nc.sync.dma_start(v_t[2:3, kv], ones)
                # v3 = (2x ox - n2) + 2y oy
                nc.vector.scalar_tensor_tensor(rtmp, r2x, ox, rn2,
                                               op0=mult, op1=sub)
                nc.vector.scalar_tensor_tensor(v3s, r2y, oy, rtmp,
                                               op0=mult, op1=add)
                nc.sync.dma_start(v_t[3:4, kv], v3s)
                nc.sync.dma_start(u_t[0:1, ku], xyi_d[0:1, :])
                nc.sync.dma_start(u_t[1:2, ku], xyi_d[1:2, :])
                nc.sync.dma_start(u_t[3:4, ku], ones[:, 0:IBLK])
                # u2 = (r2-|off|^2) - (n2_i + 2 ox x_i + 2 oy y_i)
                nc.vector.scalar_tensor_tensor(u2a, r2xi, ox, rn2i,
                                               op0=mult, op1=add)
                nc.vector.scalar_tensor_tensor(u2b, r2yi, oy, u2a,
                                               op0=mult, op1=add)
                nc.scalar.activation(u2a, u2b,
                                     mybir.ActivationFunctionType.Copy,
                                     bias=R2 - (ox * ox + oy * oy),
                                     scale=-1.0)
                nc.sync.dma_start(u_t[2:3, ku], u2a)

        # ---- phase 1: AT[j, (k,o)] = dcoef[:,j].T @ wk on-device
        at_t = const.tile([128, NCHUNK * NCELLS * C_OUT], BF16)
        with tc.tile_pool(name="atp", bufs=1, space="PSUM") as atpool:
            for c in range(NCHUNK):
                at_ps = atpool.tile([128, NCELLS * C_OUT], F32)
                # PSUM matmul outputs are limited to 512 fp32 columns (one
                # bank); split the 576-wide product
                for lo, hi in ((0, 512), (512, NCELLS * C_OUT)):
                    nc.tensor.matmul(
                        at_ps[:, lo:hi],
                        dc_t[:, c * 128 : (c + 1) * 128],
                        wk_t[:, lo:hi],
                        start=True,
                        stop=True,
                    )
                nc.scalar.copy(
                    at_t[:, c * NCELLS * C_OUT : (c + 1) * NCELLS * C_OUT],
                    at_ps[:],
                )

        tpool = ctx.enter_context(tc.tile_pool(name="t", bufs=2, space="PSUM"))

        # main-matmul accumulator: even items -> partitions 0:64,
        # odd items -> 64:128 (2-way col-tiling); halves added at the end
        out_ps = opool.tile([128, IBLK], F32)

        # ---- main loop: dense over (c, k); items paired for [128,1024]
        # relu(t)^3 ops
        items = [(c, k) for c in range(NCHUNK) for k in range(NCELLS)]
        nitems = len(items)
        for p in range(nitems // 2):
            t_ps = tpool.tile([128, 2 * IBLK], F32)
            for r in range(2):
                c, k = items[2 * p + r]
                nc.tensor.matmul(
                    t_ps[:, r * IBLK : (r + 1) * IBLK],
                    v_t[:, k * N + c * 128 : k * N + (c + 1) * 128],
                    u_t[:, k * IBLK : (k + 1) * IBLK],
                    start=True,
                    stop=True,
                )
            q_t = qpool.tile([128, 2 * IBLK], BF16)
            nc.scalar.square(q_t[:], t_ps[:])
            w_t = wpool.tile([128, 2 * IBLK], BF16)
            nc.vector.scalar_tensor_tensor(
                w_t[:], t_ps[:], 0.0, q_t[:],
                op0=mybir.AluOpType.max, op1=mybir.AluOpType.mult,
            )
            for r in range(2):
                m = 2 * p + r
                c, k = items[m]
                par = (m % 2) * C_OUT
                nc.tensor.matmul(
                    out_ps[par : par + C_OUT, :],
                    at_t[:, (c * NCELLS + k) * C_OUT : (c * NCELLS + k + 1) * C_OUT],
                    w_t[:, r * IBLK : (r + 1) * IBLK],
                    start=(m < 2),
                    stop=(m >= nitems - 2),
                    skip_group_check=True,
                    tile_position=(0, par),
                )

        tmp_sb = const.tile([C_OUT, IBLK], F32)
        nc.scalar.copy(tmp_sb[:], out_ps[0:C_OUT, :])
        out_sb = const.tile([C_OUT, IBLK], BF16)
        nc.vector.tensor_add(out_sb[:], tmp_sb[:], out_ps[C_OUT:, :])
        nc.sync.dma_start(out_d[:], out_sb[:])
    _split_sync_waits(nc)
    return nc


def _get_nc():
    if "nc" not in _cache:
        _cache["nc"] = _build_nc()
    return _cache["nc"]


# ---------------------------------------------------------- cached launcher
# run_bass_kernel_spmd rebuilds the jit closure (full retrace + XLA compile)
# on every call; cache the jitted shard_map executable instead so repeat
# launches only pay transfer + dispatch.
def _get_runner():
    if "runner" in _cache:
        return _cache["runner"]

    import jax
    from jax.sharding import Mesh, PartitionSpec
    from jax.experimental.shard_map import shard_map
    from concourse.bass2jax import (
        _bass_exec_p,
        install_neuronx_cc_hook,
        partition_id_tensor,
    )

    nc = _get_nc()
    install_neuronx_cc_hook()

    # The kernel writes every element of its outputs, so by default no
    # donated pre-zeroed output operands are shipped (saves host->device
    # bytes). KERNEL_DONATE_ZEROS=1 restores run_bass_kernel_spmd's donated
    # zero-buffer behavior as a fallback.
    donate_zeros = bool(os.environ.get("KERNEL_DONATE_ZEROS"))

    partition_name = (
        nc.partition_id_tensor.name if nc.partition_id_tensor else None
    )
    in_names, out_names, out_avals, zero_outs = [], [], [], []
    for alloc in nc.m.functions[0].allocations:
        if not isinstance(alloc, mybir.MemoryLocationSet):
            continue
        name = alloc.memorylocations[0].name
        if alloc.kind == "ExternalInput":
            if name != partition_name:
                in_names.append(name)
        elif alloc.kind == "ExternalOutput":
            shape = tuple(alloc.tensor_shape)
            dtype = mybir.dt.np(alloc.dtype)
            out_names.append(name)
            out_avals.append(jax.core.ShapedArray(shape, dtype))
            zero_outs.append(np.zeros((NCORES * shape[0], *shape[1:]), dtype))
    n_params = len(in_names)
    all_in_names = list(in_names)
    if donate_zeros:
        all_in_names += list(out_names)
    if partition_name is not None:
        all_in_names.append(partition_name)

    def _body(*args):
        operands = list(args)
        if partition_name is not None:
            operands.append(partition_id_tensor())
        return tuple(
            _bass_exec_p.bind(
                *operands,
                out_avals=tuple(out_avals),
                in_names=tuple(all_in_names),
                out_names=tuple(out_names),
                lowering_input_output_aliases=(),
                sim_require_finite=True,
                sim_require_nnan=True,
                nc=nc,
            )
        )

    devices = jax.devices()[:NCORES]
    assert len(devices) == NCORES, (
        f"need {NCORES} devices, have {len(jax.devices())}"
    )
    mesh = Mesh(np.asarray(devices), ("core",))
    n_outs = len(out_names)
    n_args = n_params + (n_outs if donate_zeros else 0)
    sharded = jax.jit(
        shard_map(
            _body,
            mesh=mesh,
            in_specs=(PartitionSpec("core"),) * n_args,
            out_specs=(PartitionSpec("core"),) * n_outs,
            check_rep=False,
        ),
        donate_argnums=(
            tuple(range(n_params, n_params + n_outs)) if donate_zeros else ()
        ),
        keep_unused=True,
    )

    runner = {
        "fn": sharded,
        "in_names": in_names,
        "out_avals": out_avals,
        "zero_outs": zero_outs if donate_zeros else [],
    }
    _cache["runner"] = runner
    return runner


def _run(by_name):
    r = _get_runner()
    concat_in = [by_name[name] for name in r["in_names"]]
    out_arrs = r["fn"](*concat_in, *r["zero_outs"])
    shape = r["out_avals"][0].shape
    return np.asarray(out_arrs[0]).reshape(NCORES, *shape)


# ------------------------------------------------------------ host wrapper
def _offsets():
    axes = [
        (np.arange(kk) - (kk - 1) / 2.0) * d for kk, d in zip(KSIZE, DILATION)
    ]
    grids = np.meshgrid(*axes, indexing="ij")
    return np.stack([g.reshape(-1) for g in grids], axis=-1).astype(np.float32)


def _prepare_in_maps(locs, data, density, weight, bias):
    """Build the per-core compact inputs, pre-concatenated along axis 0 in
    the order the runner expects (core-major)."""
    locs = np.asarray(locs, np.float32)
    data = np.asarray(data, np.float32)
    density = np.asarray(density, np.float32)
    weight = np.asarray(weight, np.float32)

    pos = locs[..., :NDIM]                       # [B,N,2]
    invmass = locs[..., NDIM]                    # [B,N]
    coef = 1.0 / (invmass * density)             # [B,N]
    dcoef = (data * coef[:, None, :]).astype(ml_dtypes.bfloat16)  # [B,C,N]
    offs = _offsets()                            # [9,2]

    # WK[c_in, k*64+o] = KNORM * weight[o, c_in, k], replicated to all cores
    wk = np.ascontiguousarray(
        (weight * KNORM).transpose(1, 2, 0).reshape(C_IN, NCELLS * C_OUT)
    ).astype(ml_dtypes.bfloat16)

    n_iblk = N // IBLK
    xy_all = np.ascontiguousarray(pos.transpose(0, 2, 1))   # [B, 2, N]
    xyi_all = np.empty((NCORES, 2, IBLK), np.float32)
    for c in range(NCORES):
        b, q = c // n_iblk, c % n_iblk
        xyi_all[c] = xy_all[b][:, q * IBLK : (q + 1) * IBLK]

    return {
        "XY": np.concatenate([xy_all[c // n_iblk] for c in range(NCORES)], 0),
        "XYI": np.concatenate(list(xyi_all), 0),
        "DC": np.concatenate([dcoef[c // n_iblk] for c in range(NCORES)], 0),
        "WK": np.concatenate([wk] * NCORES, 0),
    }


def kernel(locs, data, density, weight, bias):
    concat_in = _prepare_in_maps(locs, data, density, weight, bias)
    res = _run(concat_in)                        # [8, C_OUT, IBLK] bf16
    bias = np.asarray(bias, np.float32)
    n_iblk = N // IBLK
    out = np.empty((B, C_OUT, N), np.float32)
    for b in range(B):
        for q in range(n_iblk):
            out[b][:, q * IBLK : (q + 1) * IBLK] = res[b * n_iblk + q]
    return out + bias[None, :, None]


# -------------------------------------------------------------- benchmarking
def time_kernel(locs, data, density, weight, bias, iters=12):
    """Return (best_wall_s, per_call_s_list) for the device launch only."""
    concat_in = _prepare_in_maps(locs, data, density, weight, bias)
    _run(concat_in)  # warm (compile)
    times = []
    for _ in range(iters):
        t0 = time.perf_counter()
        _run(concat_in)
        times.append(time.perf_counter() - t0)
    return min(times), times
